# revision 2
# baseline (speedup 1.0000x reference)
"""2-layer GAT on 8 Trainium2 NeuronCores — v2 (dma_gather batch gathers).

Design (vs v1 per-block indirect DMAs):
  - One fused table per layer, [NP, 128] bf16-typed 256B rows:
      L1 row: [h bf16 x64 | sS f32 x8 (slots 64..80) | sD f32 x8 (80..96) | pad]
      L2 row: [h bf16 x64 | sS f32 x1 (64..66) | sD f32 x1 (66..68) | pad]
    f32 scores are bit-embedded in the bf16 tensor and read via AP bitcast.
  - Nodes are slot-permuted (perm) so both layers share one set of edge
    indices; dst rows for a core's windows are its OWN rows, so the dst
    (sD) gather reads the core-local pre-collective table t{1,2}s_d with
    local int16 indices — no AllGather dependency, hoisted before it.
  - src gathers use dma_gather (int16 idx) split into A (<32768) / B runs
    of blocks per super; per-slot PSUM matmul accumulation as in v1.
  - Scores f32 end-to-end (f32 node matmul, f32 e/lrelu), p bf16.
  - Output bf16, converted to f32 on host.
"""

import numpy as np
import ml_dtypes

P = 128
W = 64
NC = 8
WPC = 98
NPC = WPC * W          # 6272
NP = NC * NPC          # 50176
HALF = 32768
IN_DIM = 128
HEADS1 = 8
OUT_DIM = 64
NEG_SLOPE = 0.2
SUPER_BLK = 72
SRCW = 80              # bf16 slots gathered per src row (h64 + sS f32x8)
DSTW = 16              # bf16 slots gathered per dst row (32B)
EW = 128               # table row width in bf16 slots (256B stride)


def _mk_head_mat(a):
    H, C = a.shape
    A = np.zeros((H * C, H), np.float32)
    for h in range(H):
        A[h * C:(h + 1) * C, h] = a[h]
    return A


def _wrap16(vals):
    """int16 index stream [n*128] -> wrapped [16, n*8] (i at [i%16, i//16])."""
    v = np.asarray(vals, np.int16)
    assert len(v) % 16 == 0
    return np.ascontiguousarray(v.reshape(-1, 16).T)


def _prep(x, edge_index, W1, a_src1, a_dst1, b1, W2, a_src2, a_dst2, b2):
    n = x.shape[0]
    assert n <= NP

    x = np.asarray(x, np.float32)
    ei = np.asarray(edge_index)
    src = np.concatenate([ei[0], np.arange(n)]).astype(np.int64)
    dst = np.concatenate([ei[1], np.arange(n)]).astype(np.int64)

    win = (dst // W).astype(np.int64)
    order = np.argsort(win, kind="stable")
    src, dst, win = src[order], dst[order], win[order]
    nw = NP // W
    counts = np.bincount(win, minlength=nw)
    starts = np.concatenate([[0], np.cumsum(counts)])

    counts_c = counts.reshape(NC, WPC)
    orders = [np.argsort(-counts_c[c], kind="stable") for c in range(NC)]

    # perm[node] = row in slot-ordered table (slot s of core c holds window
    # orders[c][s]);  pinv[row] = node
    perm = np.empty(NP, np.int64)
    for c in range(NC):
        inv = np.empty(WPC, np.int64)
        inv[orders[c]] = np.arange(WPC)
        wl = np.arange(WPC)
        base = (c * WPC + wl) * W
        for woff in range(W):
            perm[base + woff] = c * NPC + inv * W + woff
    pinv = np.empty(NP, np.int64)
    pinv[perm] = np.arange(NP)

    psrc = perm[src]
    pdst = perm[dst]
    isA = psrc < HALF

    # per (core, slot) A/B edge counts -> uniform KA/KB
    nA = np.zeros((NC, WPC), np.int64)
    nB = np.zeros((NC, WPC), np.int64)
    for c in range(NC):
        for s in range(WPC):
            wglob = c * WPC + orders[c][s]
            e0, e1 = starts[wglob], starts[wglob + 1]
            a = int(isA[e0:e1].sum())
            nA[c, s] = a
            nB[c, s] = (e1 - e0) - a
    KA = np.maximum(np.ceil(nA / P).astype(np.int64).max(axis=0), 0)
    KB = np.ceil(nB / P).astype(np.int64).max(axis=0)
    # every slot needs >=1 block for psum start/stop (self loops guarantee
    # edges exist, but they might all be in one half)
    KA = np.where(KA + KB == 0, 1, KA)
    Mtot = int((KA + KB).sum())

    # greedy supers
    supers = []   # dicts: sl0, nsl, bb0, nA (blocks), nB, slotsA, slotsB
    s = 0
    bb0 = 0
    while s < WPC:
        sl0 = s
        nblk = 0
        while s < WPC and nblk + KA[s] + KB[s] <= SUPER_BLK:
            nblk += int(KA[s] + KB[s])
            s += 1
        nsl = s - sl0
        nAs = int(KA[sl0:s].sum())
        nBs = int(KB[sl0:s].sum())
        slotsA, slotsB = [], []
        ao, bo = 0, nAs
        for k in range(nsl):
            slotsA.append(list(range(ao, ao + int(KA[sl0 + k]))))
            slotsB.append(list(range(bo, bo + int(KB[sl0 + k]))))
            ao += int(KA[sl0 + k])
            bo += int(KB[sl0 + k])
        supers.append(dict(sl0=sl0, nsl=nsl, bb0=bb0, nA=nAs, nB=nBs,
                           slotsA=slotsA, slotsB=slotsB))
        bb0 += nAs + nBs
    assert bb0 == Mtot

    # per-core streams
    in_maps = []
    for c in range(NC):
        isrc_cols = np.zeros((16, Mtot * 8), np.int16)
        idst_cols = np.zeros((16, Mtot * 8), np.int16)
        edstl = np.full((P, Mtot), W, ml_dtypes.bfloat16)
        for S in supers:
            sl0, nsl, b0 = S["sl0"], S["nsl"], S["bb0"]
            sA = np.zeros(S["nA"] * P, np.int64)   # src rel (A)
            sB = np.zeros(S["nB"] * P, np.int64)
            dA = np.zeros(S["nA"] * P, np.int64)   # dst rel (local)
            dB = np.zeros(S["nB"] * P, np.int64)
            lA = np.full(S["nA"] * P, W, np.int64)
            lB = np.full(S["nB"] * P, W, np.int64)
            ao = bo = 0
            for k in range(nsl):
                sidx = sl0 + k
                wglob = c * WPC + orders[c][sidx]
                e0, e1 = starts[wglob], starts[wglob + 1]
                m = isA[e0:e1]
                pse, pde, de = psrc[e0:e1], pdst[e0:e1], dst[e0:e1]
                ea, eb = m.nonzero()[0], (~m).nonzero()[0]
                na, nb_ = len(ea), len(eb)
                sA[ao:ao + na] = pse[ea]
                dA[ao:ao + na] = pde[ea] - c * NPC
                lA[ao:ao + na] = de[ea] - wglob * W
                sB[bo:bo + nb_] = pse[eb] - HALF
                dB[bo:bo + nb_] = pde[eb] - c * NPC
                lB[bo:bo + nb_] = de[eb] - wglob * W
                ao += int(KA[sidx]) * P
                bo += int(KB[sidx]) * P
            assert dA.max(initial=0) < NPC and dB.max(initial=0) < NPC
            stream_l = np.concatenate([lA, lB])
            nblk = S["nA"] + S["nB"]
            edstl[:, b0:b0 + nblk] = (
                stream_l.reshape(nblk, P).T.astype(ml_dtypes.bfloat16))
            isrc_cols[:, 8 * b0:8 * (b0 + S["nA"])] = _wrap16(sA)
            isrc_cols[:, 8 * (b0 + S["nA"]):8 * (b0 + nblk)] = _wrap16(sB)
            idst_cols[:, 8 * b0:8 * (b0 + nblk)] = _wrap16(
                np.concatenate([dA, dB]))

        xp = np.zeros((NPC, IN_DIM), np.float32)
        rows = pinv[c * NPC:(c + 1) * NPC]
        valid = rows < n
        xp[valid] = x[rows[valid]]
        in_maps.append({
            "xT": np.ascontiguousarray(xp.T),
            "isrc": np.ascontiguousarray(isrc_cols),
            "idst": np.ascontiguousarray(idst_cols),
            "edstl": np.ascontiguousarray(edstl),
        })

    W1 = np.asarray(W1, np.float32)
    W2 = np.asarray(W2, np.float32)
    wc1 = np.concatenate(
        [W1, W1 @ _mk_head_mat(np.asarray(a_src1, np.float32)),
         W1 @ _mk_head_mat(np.asarray(a_dst1, np.float32))], axis=1)  # [128,80]
    wc2 = np.concatenate(
        [W2, W2 @ np.asarray(a_src2, np.float32).T,
         W2 @ np.asarray(a_dst2, np.float32).T], axis=1)              # [64,66]
    b1r = np.tile(np.asarray(b1, np.float32)[None, :], (W, 1))
    b2r = np.tile(np.asarray(b2, np.float32)[None, :], (W, 1))
    for m in in_maps:
        m["wc1"] = np.ascontiguousarray(wc1, np.float32)
        m["wc2"] = np.ascontiguousarray(wc2, np.float32)
        m["b1r"] = np.ascontiguousarray(b1r, np.float32)
        m["b2r"] = np.ascontiguousarray(b2r, np.float32)

    cfg = dict(Mtot=Mtot, supers=supers)
    return cfg, in_maps, perm


def _sub(apbase, off, dims):
    import concourse.bass as bass
    return bass.AP(tensor=apbase.tensor, offset=apbase.offset + off,
                   ap=[list(apbase.ap[0])] + [list(d) for d in dims])


def _dram_ap(tensor, off, dims):
    import concourse.bass as bass
    return bass.AP(tensor=tensor, offset=off, ap=[list(d) for d in dims])


def _gather(g, out_ap, in_ap, idxs_ap, num_idxs, elem_size, reg):
    """dma_gather without the elem_size%256 assert (sub-256B reads verified
    on hw)."""
    import concourse.mybir as mybir
    elem_step = in_ap.ap[0][0]
    stride_bytes = elem_step * mybir.dt.size(in_ap.dtype)
    assert stride_bytes % 256 == 0
    _in_ap = g.lower_ap_dma(in_ap, for_custom_bir_dma=True)
    _idxs_ap = g.lower_ap(idxs_ap)
    _out_ap = g.lower_ap(out_ap)
    return g.add_instruction(
        mybir.InstDMAGatherAnt(
            name=g.bass.get_next_instruction_name(),
            ins=[*_in_ap, _idxs_ap, g.lower_val_access(reg)],
            outs=[_out_ap],
            transpose=False, num_idxs=num_idxs, elem_size=elem_size,
            stride_bytes_256=stride_bytes // 256, gen_mode=0,
            single_packet=True, queue_num=0, sbuf_tokens_per_rank=0,
            sbuf_free_dim_per_rank=0, sbuf_free_dim_pad_per_rank=0,
            sbuf_byte_offset=0,
        )
    )


def _build(nc, cfg):
    import concourse.mybir as mybir
    import concourse.tile as tile
    import concourse.library_config as library_config

    f32 = mybir.dt.float32
    bf16 = mybir.dt.bfloat16
    i16 = mybir.dt.int16
    Alu = mybir.AluOpType
    Act = mybir.ActivationFunctionType

    Mtot = cfg["Mtot"]
    supers = cfg["supers"]
    groups = [list(range(NC))]
    nt = NPC // P   # 49 node tiles per core

    xT_d = nc.dram_tensor("xT", [IN_DIM, NPC], f32, kind="ExternalInput")
    isrc_d = nc.dram_tensor("isrc", [16, Mtot * 8], i16, kind="ExternalInput")
    idst_d = nc.dram_tensor("idst", [16, Mtot * 8], i16, kind="ExternalInput")
    edstl_d = nc.dram_tensor("edstl", [P, Mtot], bf16, kind="ExternalInput")
    wc1_d = nc.dram_tensor("wc1", [IN_DIM, 80], f32, kind="ExternalInput")
    wc2_d = nc.dram_tensor("wc2", [64, 66], f32, kind="ExternalInput")
    b1r_d = nc.dram_tensor("b1r", [W, 64], f32, kind="ExternalInput")
    b2r_d = nc.dram_tensor("b2r", [W, 64], f32, kind="ExternalInput")
    out_d = nc.dram_tensor("out", [NPC, OUT_DIM], bf16, kind="ExternalOutput")

    import os as _os
    _dbg = bool(_os.environ.get("K_DEBUG"))
    if _dbg:
        dbg1_d = nc.dram_tensor("dbg1", [NPC, EW], bf16, kind="ExternalOutput")
        dbg2_d = nc.dram_tensor("dbg2", [W, WPC * W], f32, kind="ExternalOutput")
    t1s_d = nc.dram_tensor("t1slice", [NPC, EW], bf16, kind="Internal")
    table1 = nc.dram_tensor("table1", [NP, EW], bf16, kind="Internal",
                            addr_space="Shared")
    t2s_d = nc.dram_tensor("t2slice", [NPC, EW], bf16, kind="Internal")
    table2 = nc.dram_tensor("table2", [NP, EW], bf16, kind="Internal",
                            addr_space="Shared")

    iota_d = nc.inline_tensor(
        np.tile(np.arange(W).astype(ml_dtypes.bfloat16), (P, 1)), "iotaw")
    ident_d = nc.inline_tensor(np.eye(W, dtype=np.float32), "identw")

    with tile.TileContext(nc) as tc:
        with tc.tile_pool(name="const", bufs=1) as cp, \
             tc.tile_pool(name="work", bufs=3) as wp, \
             tc.tile_pool(name="gath", bufs=2) as gp, \
             tc.tile_pool(name="dstg", bufs=1) as dgp, \
             tc.tile_pool(name="ohp", bufs=2) as op_, \
             tc.tile_pool(name="drain", bufs=3) as dp, \
             tc.tile_pool(name="eps", bufs=3, space="PSUM") as pp, \
             tc.tile_pool(name="nps", bufs=2, space="PSUM") as np_, \
             tc.tile_pool(name="tps", bufs=1, space="PSUM") as tpp:

            nc.gpsimd.load_library(library_config.mlp)

            iota = cp.tile([P, W], bf16, tag="iota")
            nc.scalar.dma_start(out=iota[:, :], in_=iota_d[:, :])
            ident = cp.tile([W, W], f32, tag="ident")
            nc.scalar.dma_start(out=ident[:, :], in_=ident_d[:, :])
            wc1 = cp.tile([IN_DIM, 80], f32, tag="wc1")
            nc.scalar.dma_start(out=wc1[:, :], in_=wc1_d[:, :])
            wc2 = cp.tile([64, 66], f32, tag="wc2")
            nc.scalar.dma_start(out=wc2[:, :], in_=wc2_d[:, :])
            b1r = cp.tile([W, 64], f32, tag="b1r")
            nc.scalar.dma_start(out=b1r[:, :], in_=b1r_d[:, :])
            b2r = cp.tile([W, 64], f32, tag="b2r")
            nc.scalar.dma_start(out=b2r[:, :], in_=b2r_d[:, :])
            edstl = cp.tile([P, Mtot], bf16, tag="edstl")
            nc.scalar.dma_start(out=edstl[:, :], in_=edstl_d[:, :])
            isrc = cp.tile([P, Mtot * 8], i16, tag="isrc")
            idst = cp.tile([P, Mtot * 8], i16, tag="idst")
            for k in range(8):
                nc.scalar.dma_start(out=isrc[16 * k:16 * (k + 1), :],
                                    in_=isrc_d[:, :])
                nc.scalar.dma_start(out=idst[16 * k:16 * (k + 1), :],
                                    in_=idst_d[:, :])
            h2big = cp.tile([W, WPC * W], f32, tag="h2big")

            # distinct num_idxs register values (chunks are <=8 blocks)
            regs = {}
            for S in supers:
                for run in (S["nA"], S["nB"], S["nA"] + S["nB"]):
                    left = run
                    while left > 0:
                        nb = min(8, left)
                        if nb * P not in regs:
                            regs[nb * P] = nc.gpsimd.to_reg(nb * P)
                        left -= nb

            def node_phase1():
                stage = cp.tile([P, nt * EW], bf16, tag="nstage")
                stf = stage[:, :].bitcast(f32)
                for t in range(nt):
                    xt = wp.tile([IN_DIM, P], f32, tag="xt")
                    nc.sync.dma_start(out=xt[:, :],
                                      in_=xT_d[:, t * P:(t + 1) * P])
                    hp = np_.tile([P, 80], f32, tag="hps")
                    nc.tensor.matmul(out=hp[:, :], lhsT=xt[:, :],
                                     rhs=wc1[:, :], start=True, stop=True)
                    nc.vector.tensor_copy(
                        out=stage[:, t * EW:t * EW + 64], in_=hp[:, :64])
                    nc.vector.tensor_copy(
                        out=_sub(stf, t * (EW // 2) + 32, [[1, 16]]),
                        in_=hp[:, 64:80])
                # stage [p, t*EW+slot] -> t1s row t*128+p
                nc.sync.dma_start(
                    out=_dram_ap(t1s_d, 0,
                                 [[EW, P], [EW * P, nt], [1, EW]]),
                    in_=_sub(stage[:, :], 0, [[EW, nt], [1, EW]]))
                nc.gpsimd.collective_compute(
                    "AllGather", Alu.bypass, replica_groups=groups,
                    ins=[t1s_d[:, :]], outs=[table1[:, :]])

            def node_phase2():
                stage = cp.tile([P, nt * EW], bf16, tag="nstage")
                stf = stage[:, :].bitcast(f32)
                for t in range(nt):
                    tp2 = tpp.tile([W, P], f32, tag="tps")
                    nc.tensor.transpose(tp2[:, :W],
                                        h2big[:, (2 * t) * W:(2 * t + 1) * W],
                                        ident[:, :])
                    nc.tensor.transpose(tp2[:, W:],
                                        h2big[:, (2 * t + 1) * W:(2 * t + 2) * W],
                                        ident[:, :])
                    h2T = wp.tile([W, P], f32, tag="h2T")
                    nc.vector.tensor_copy(out=h2T[:, :], in_=tp2[:, :])
                    hp2 = np_.tile([P, 66], f32, tag="hp2s")
                    nc.tensor.matmul(out=hp2[:, :], lhsT=h2T[:, :],
                                     rhs=wc2[:, :], start=True, stop=True)
                    nc.vector.tensor_copy(
                        out=stage[:, t * EW:t * EW + 64], in_=hp2[:, :64])
                    nc.vector.tensor_copy(
                        out=_sub(stf, t * (EW // 2) + 32, [[1, 2]]),
                        in_=hp2[:, 64:66])
                nc.sync.dma_start(
                    out=_dram_ap(t2s_d, 0,
                                 [[EW, P], [EW * P, nt], [1, EW]]),
                    in_=_sub(stage[:, :], 0, [[EW, nt], [1, EW]]))
                nc.gpsimd.collective_compute(
                    "AllGather", Alu.bypass, replica_groups=groups,
                    ins=[t2s_d[:, :]], outs=[table2[:, :]])

            GMAX = 8    # max blocks (1024 idxs) per dma_gather (hw ring cap)

            def chunked_gather(dst_tile, dst_w, blk0, nblk, src_ap, idx_tile,
                               icol0):
                """gather nblk blocks into dst_tile cols [blk0*dst_w ...),
                chunks of <=GMAX blocks; idx cols start at icol0."""
                done = 0
                while done < nblk:
                    nb = min(GMAX, nblk - done)
                    _gather(nc.gpsimd,
                            _sub(dst_tile[:, :], (blk0 + done) * dst_w,
                                 [[dst_w, nb], [1, dst_w]]),
                            src_ap,
                            idx_tile[:, icol0 + 8 * done:
                                     icol0 + 8 * (done + nb)],
                            nb * P, dst_w, regs[nb * P])
                    done += nb

            def edge_phase(table, loc_table, H, GW_mm, dstoff, doff, layer):
                # dst gathers first: independent of the AllGather
                Dts = []
                for sp, S in enumerate(supers):
                    nblk = S["nA"] + S["nB"]
                    D = dgp.tile([P, nblk * DSTW], bf16, tag=f"D{sp}")
                    chunked_gather(D, DSTW, 0, nblk,
                                   loc_table[:, dstoff:dstoff + DSTW],
                                   idst, 8 * S["bb0"])
                    Dts.append(D)

                for sp, S in enumerate(supers):
                    sl0, nsl, bb0 = S["sl0"], S["nsl"], S["bb0"]
                    nAs, nBs = S["nA"], S["nB"]
                    nblk = nAs + nBs
                    G = gp.tile([P, nblk * SRCW], bf16, tag="G")
                    if nAs:
                        chunked_gather(G, SRCW, 0, nAs,
                                       table[:HALF, :SRCW], isrc, 8 * bb0)
                    if nBs:
                        chunked_gather(G, SRCW, nAs, nBs,
                                       table[HALF:NP, :SRCW], isrc,
                                       8 * (bb0 + nAs))
                    D = Dts[sp]
                    Gf = G[:, :].bitcast(f32)       # [P, nblk*40]
                    Df = D[:, :].bitcast(f32)       # [P, nblk*8]
                    e = wp.tile([P, nblk * H], f32, tag="e")
                    nc.vector.tensor_tensor(
                        out=_sub(e[:, :], 0, [[H, nblk], [1, H]]),
                        in0=_sub(Gf, 32, [[SRCW // 2, nblk], [1, H]]),
                        in1=_sub(Df, doff, [[DSTW // 2, nblk], [1, H]]),
                        op=Alu.add)
                    # lrelu = max(0.2*e, e); scalar-engine Lrelu ignores alpha
                    nc.vector.scalar_tensor_tensor(
                        out=e[:, :], in0=e[:, :], scalar=NEG_SLOPE,
                        in1=e[:, :], op0=Alu.mult, op1=Alu.max)
                    nc.scalar.activation(
                        out=_sub(G[:, :], 64, [[SRCW, nblk], [1, H]]),
                        in_=_sub(e[:, :], 0, [[H, nblk], [1, H]]),
                        func=Act.Exp)
                    oh = op_.tile([P, nblk * W], bf16, tag="oh")
                    nc.vector.tensor_tensor(
                        out=_sub(oh[:, :], 0, [[W, nblk], [1, W]]),
                        in0=_sub(iota[:, :], 0, [[0, nblk], [1, W]]),
                        in1=_sub(edstl[:, :], bb0, [[1, nblk], [0, W]]),
                        op=Alu.is_equal)
                    in0m = _sub(G[:, :], 0,
                                [[SRCW, nblk], [64 // H, H], [1, 64 // H]])
                    in1p = _sub(G[:, :], 64,
                                [[SRCW, nblk], [1, H], [0, 64 // H]])
                    nc.vector.tensor_tensor(out=in0m, in0=in0m, in1=in1p,
                                            op=Alu.mult)

                    stage = dp.tile([W, nsl * 64], f32, tag="stage")
                    for k in range(nsl):
                        pos = S["slotsA"][k] + S["slotsB"][k]
                        ps = pp.tile([W, GW_mm], f32, tag="ps")
                        for i, j in enumerate(pos):
                            nc.tensor.matmul(
                                out=ps[:, :],
                                lhsT=oh[:, j * W:(j + 1) * W],
                                rhs=G[:, j * SRCW:j * SRCW + GW_mm],
                                start=(i == 0), stop=(i == len(pos) - 1))
                        den = dp.tile([W, H], f32, tag="den")
                        nc.vector.tensor_scalar_add(den[:, :],
                                                    ps[:, 64:64 + H], 1e-10)
                        inv = dp.tile([W, H], f32, tag="inv")
                        nc.vector.reciprocal(inv[:, :], den[:, :])
                        nc.vector.tensor_tensor(
                            out=_sub(stage[:, :], k * 64,
                                     [[64 // H, H], [1, 64 // H]]),
                            in0=_sub(ps[:, :], 0,
                                     [[64 // H, H], [1, 64 // H]]),
                            in1=_sub(inv[:, :], 0, [[1, H], [0, 64 // H]]),
                            op=Alu.mult)
                    if layer == 1:
                        tb = dp.tile([W, nsl * 64], f32, tag="tb")
                        nc.vector.tensor_tensor(
                            out=tb[:, :], in0=stage[:, :],
                            in1=_sub(b1r[:, :], 0, [[0, nsl], [1, 64]]),
                            op=Alu.add)
                        ex = dp.tile([W, nsl * 64], f32, tag="ex")
                        nc.scalar.activation(out=ex[:, :], in_=tb[:, :],
                                             func=Act.Exp)
                        nc.vector.tensor_scalar(
                            out=ex[:, :], in0=ex[:, :], scalar1=-1.0,
                            scalar2=0.0, op0=Alu.add, op1=Alu.min)
                        rl = dp.tile([W, nsl * 64], f32, tag="rl")
                        nc.vector.tensor_scalar_max(rl[:, :], tb[:, :], 0.0)
                        nc.vector.tensor_tensor(
                            out=h2big[:, sl0 * 64:(sl0 + nsl) * 64],
                            in0=ex[:, :], in1=rl[:, :], op=Alu.add)
                    else:
                        ob = dp.tile([W, nsl * 64], bf16, tag="ob")
                        nc.vector.tensor_tensor(
                            out=ob[:, :], in0=stage[:, :],
                            in1=_sub(b2r[:, :], 0, [[0, nsl], [1, 64]]),
                            op=Alu.add)
                        # ob[woff, k*64+c] -> out row (sl0+k)*64+woff
                        nc.sync.dma_start(
                            out=_dram_ap(out_d, sl0 * W * 64,
                                         [[64, W], [W * 64, nsl], [1, 64]]),
                            in_=_sub(ob[:, :], 0, [[64, nsl], [1, 64]]))

            node_phase1()
            if _dbg:
                nc.sync.dma_start(out=dbg1_d[:, :], in_=t1s_d[:, :])
            edge_phase(table1, t1s_d, HEADS1, 72, 80, 0, layer=1)
            if _dbg:
                nc.sync.dma_start(out=dbg2_d[:, :], in_=h2big[:, :])
            node_phase2()
            edge_phase(table2, t2s_d, 1, 65, 64, 1, layer=2)


def kernel(**inputs):
    import concourse.bacc as bacc
    from concourse.bass_utils import run_bass_kernel_spmd

    n = inputs["x"].shape[0]
    cfg, in_maps, perm = _prep(**inputs)

    nc = bacc.Bacc("TRN2", target_bir_lowering=False, debug=False,
                   num_devices=NC)
    _build(nc, cfg)
    nc.compile()

    res = run_bass_kernel_spmd(nc, in_maps, core_ids=list(range(NC)))
    full = np.concatenate([np.asarray(r["out"]) for r in res.results], axis=0)
    out = full[perm[:n]].astype(np.float32)
    return np.ascontiguousarray(out)


# revision 3
# speedup vs baseline: 1.0049x; 1.0049x over previous
"""2-layer GAT on 8 Trainium2 NeuronCores — v2 (dma_gather batch gathers).

Design (vs v1 per-block indirect DMAs):
  - One fused table per layer, [NP, 128] bf16-typed 256B rows:
      L1 row: [h bf16 x64 | sS f32 x8 (slots 64..80) | sD f32 x8 (80..96) | pad]
      L2 row: [h bf16 x64 | sS f32 x1 (64..66) | sD f32 x1 (66..68) | pad]
    f32 scores are bit-embedded in the bf16 tensor and read via AP bitcast.
  - Nodes are slot-permuted (perm) so both layers share one set of edge
    indices; dst rows for a core's windows are its OWN rows, so the dst
    (sD) gather reads the core-local pre-collective table t{1,2}s_d with
    local int16 indices — no AllGather dependency, hoisted before it.
  - src gathers use dma_gather (int16 idx) split into A (<32768) / B runs
    of blocks per super; per-slot PSUM matmul accumulation as in v1.
  - Scores f32 end-to-end (f32 node matmul, f32 e/lrelu), p bf16.
  - Output bf16, converted to f32 on host.
"""

import numpy as np
import ml_dtypes

P = 128
W = 64
NC = 8
WPC = 98
NPC = WPC * W          # 6272
NP = NC * NPC          # 50176
HALF = 32768
IN_DIM = 128
HEADS1 = 8
OUT_DIM = 64
NEG_SLOPE = 0.2
SUPER_BLK = 72
SRCW = 80              # bf16 slots gathered per src row (h64 + sS f32x8)
DSTW = 16              # bf16 slots gathered per dst row (32B)
EW = 128               # table row width in bf16 slots (256B stride)


def _mk_head_mat(a):
    H, C = a.shape
    A = np.zeros((H * C, H), np.float32)
    for h in range(H):
        A[h * C:(h + 1) * C, h] = a[h]
    return A


def _wrap16(vals):
    """int16 index stream [n*128] -> wrapped [16, n*8] (i at [i%16, i//16])."""
    v = np.asarray(vals, np.int16)
    assert len(v) % 16 == 0
    return np.ascontiguousarray(v.reshape(-1, 16).T)


def _prep(x, edge_index, W1, a_src1, a_dst1, b1, W2, a_src2, a_dst2, b2):
    n = x.shape[0]
    assert n <= NP

    x = np.asarray(x, np.float32)
    ei = np.asarray(edge_index)
    src = np.concatenate([ei[0], np.arange(n)]).astype(np.int64)
    dst = np.concatenate([ei[1], np.arange(n)]).astype(np.int64)

    win = (dst // W).astype(np.int64)
    order = np.argsort(win, kind="stable")
    src, dst, win = src[order], dst[order], win[order]
    nw = NP // W
    counts = np.bincount(win, minlength=nw)
    starts = np.concatenate([[0], np.cumsum(counts)])

    counts_c = counts.reshape(NC, WPC)
    orders = [np.argsort(-counts_c[c], kind="stable") for c in range(NC)]

    # perm[node] = row in slot-ordered table (slot s of core c holds window
    # orders[c][s]);  pinv[row] = node
    perm = np.empty(NP, np.int64)
    for c in range(NC):
        inv = np.empty(WPC, np.int64)
        inv[orders[c]] = np.arange(WPC)
        wl = np.arange(WPC)
        base = (c * WPC + wl) * W
        for woff in range(W):
            perm[base + woff] = c * NPC + inv * W + woff
    pinv = np.empty(NP, np.int64)
    pinv[perm] = np.arange(NP)

    psrc = perm[src]
    pdst = perm[dst]
    isA = psrc < HALF

    # per (core, slot) A/B edge counts -> uniform KA/KB
    nA = np.zeros((NC, WPC), np.int64)
    nB = np.zeros((NC, WPC), np.int64)
    for c in range(NC):
        for s in range(WPC):
            wglob = c * WPC + orders[c][s]
            e0, e1 = starts[wglob], starts[wglob + 1]
            a = int(isA[e0:e1].sum())
            nA[c, s] = a
            nB[c, s] = (e1 - e0) - a
    KA = np.maximum(np.ceil(nA / P).astype(np.int64).max(axis=0), 0)
    KB = np.ceil(nB / P).astype(np.int64).max(axis=0)
    # every slot needs >=1 block for psum start/stop (self loops guarantee
    # edges exist, but they might all be in one half)
    KA = np.where(KA + KB == 0, 1, KA)
    Mtot = int((KA + KB).sum())

    # greedy supers
    supers = []   # dicts: sl0, nsl, bb0, nA (blocks), nB, slotsA, slotsB
    s = 0
    bb0 = 0
    while s < WPC:
        sl0 = s
        nblk = 0
        while s < WPC and nblk + KA[s] + KB[s] <= SUPER_BLK:
            nblk += int(KA[s] + KB[s])
            s += 1
        nsl = s - sl0
        nAs = int(KA[sl0:s].sum())
        nBs = int(KB[sl0:s].sum())
        slotsA, slotsB = [], []
        ao, bo = 0, nAs
        for k in range(nsl):
            slotsA.append(list(range(ao, ao + int(KA[sl0 + k]))))
            slotsB.append(list(range(bo, bo + int(KB[sl0 + k]))))
            ao += int(KA[sl0 + k])
            bo += int(KB[sl0 + k])
        supers.append(dict(sl0=sl0, nsl=nsl, bb0=bb0, nA=nAs, nB=nBs,
                           slotsA=slotsA, slotsB=slotsB))
        bb0 += nAs + nBs
    assert bb0 == Mtot

    # per-core streams
    in_maps = []
    for c in range(NC):
        isrc_cols = np.zeros((16, Mtot * 8), np.int16)
        idst_cols = np.zeros((16, Mtot * 8), np.int16)
        edstl = np.full((P, Mtot), W, ml_dtypes.bfloat16)
        for S in supers:
            sl0, nsl, b0 = S["sl0"], S["nsl"], S["bb0"]
            sA = np.zeros(S["nA"] * P, np.int64)   # src rel (A)
            sB = np.zeros(S["nB"] * P, np.int64)
            dA = np.zeros(S["nA"] * P, np.int64)   # dst rel (local)
            dB = np.zeros(S["nB"] * P, np.int64)
            lA = np.full(S["nA"] * P, W, np.int64)
            lB = np.full(S["nB"] * P, W, np.int64)
            ao = bo = 0
            for k in range(nsl):
                sidx = sl0 + k
                wglob = c * WPC + orders[c][sidx]
                e0, e1 = starts[wglob], starts[wglob + 1]
                m = isA[e0:e1]
                pse, pde, de = psrc[e0:e1], pdst[e0:e1], dst[e0:e1]
                ea, eb = m.nonzero()[0], (~m).nonzero()[0]
                na, nb_ = len(ea), len(eb)
                sA[ao:ao + na] = pse[ea]
                dA[ao:ao + na] = pde[ea] - c * NPC
                lA[ao:ao + na] = de[ea] - wglob * W
                sB[bo:bo + nb_] = pse[eb] - HALF
                dB[bo:bo + nb_] = pde[eb] - c * NPC
                lB[bo:bo + nb_] = de[eb] - wglob * W
                ao += int(KA[sidx]) * P
                bo += int(KB[sidx]) * P
            assert dA.max(initial=0) < NPC and dB.max(initial=0) < NPC
            stream_l = np.concatenate([lA, lB])
            nblk = S["nA"] + S["nB"]
            edstl[:, b0:b0 + nblk] = (
                stream_l.reshape(nblk, P).T.astype(ml_dtypes.bfloat16))
            isrc_cols[:, 8 * b0:8 * (b0 + S["nA"])] = _wrap16(sA)
            isrc_cols[:, 8 * (b0 + S["nA"]):8 * (b0 + nblk)] = _wrap16(sB)
            idst_cols[:, 8 * b0:8 * (b0 + nblk)] = _wrap16(
                np.concatenate([dA, dB]))

        xp = np.zeros((NPC, IN_DIM), np.float32)
        rows = pinv[c * NPC:(c + 1) * NPC]
        valid = rows < n
        xp[valid] = x[rows[valid]]
        in_maps.append({
            "xT": np.ascontiguousarray(xp.T.astype(np.float16)),
            "isrc": np.ascontiguousarray(isrc_cols),
            "idst": np.ascontiguousarray(idst_cols),
            "edstl": np.ascontiguousarray(edstl),
        })

    W1 = np.asarray(W1, np.float32)
    W2 = np.asarray(W2, np.float32)
    wc1 = np.concatenate(
        [W1, W1 @ _mk_head_mat(np.asarray(a_src1, np.float32)),
         W1 @ _mk_head_mat(np.asarray(a_dst1, np.float32))], axis=1)  # [128,80]
    wc2 = np.concatenate(
        [W2, W2 @ np.asarray(a_src2, np.float32).T,
         W2 @ np.asarray(a_dst2, np.float32).T], axis=1)              # [64,66]
    b1r = np.tile(np.asarray(b1, np.float32)[None, :], (W, 1))
    b2r = np.tile(np.asarray(b2, np.float32)[None, :], (W, 1))
    for m in in_maps:
        m["wc1"] = np.ascontiguousarray(wc1, np.float32)
        m["wc2"] = np.ascontiguousarray(wc2, np.float32)
        m["b1r"] = np.ascontiguousarray(b1r, np.float32)
        m["b2r"] = np.ascontiguousarray(b2r, np.float32)

    cfg = dict(Mtot=Mtot, supers=supers)
    return cfg, in_maps, perm


def _sub(apbase, off, dims):
    import concourse.bass as bass
    return bass.AP(tensor=apbase.tensor, offset=apbase.offset + off,
                   ap=[list(apbase.ap[0])] + [list(d) for d in dims])


def _dram_ap(tensor, off, dims):
    import concourse.bass as bass
    return bass.AP(tensor=tensor, offset=off, ap=[list(d) for d in dims])


def _gather(g, out_ap, in_ap, idxs_ap, num_idxs, elem_size, reg):
    """dma_gather without the elem_size%256 assert (sub-256B reads verified
    on hw)."""
    import concourse.mybir as mybir
    elem_step = in_ap.ap[0][0]
    stride_bytes = elem_step * mybir.dt.size(in_ap.dtype)
    assert stride_bytes % 256 == 0
    _in_ap = g.lower_ap_dma(in_ap, for_custom_bir_dma=True)
    _idxs_ap = g.lower_ap(idxs_ap)
    _out_ap = g.lower_ap(out_ap)
    return g.add_instruction(
        mybir.InstDMAGatherAnt(
            name=g.bass.get_next_instruction_name(),
            ins=[*_in_ap, _idxs_ap, g.lower_val_access(reg)],
            outs=[_out_ap],
            transpose=False, num_idxs=num_idxs, elem_size=elem_size,
            stride_bytes_256=stride_bytes // 256, gen_mode=0,
            single_packet=True, queue_num=0, sbuf_tokens_per_rank=0,
            sbuf_free_dim_per_rank=0, sbuf_free_dim_pad_per_rank=0,
            sbuf_byte_offset=0,
        )
    )


def _build(nc, cfg):
    import concourse.mybir as mybir
    import concourse.tile as tile
    import concourse.library_config as library_config

    f32 = mybir.dt.float32
    bf16 = mybir.dt.bfloat16
    i16 = mybir.dt.int16
    Alu = mybir.AluOpType
    Act = mybir.ActivationFunctionType

    Mtot = cfg["Mtot"]
    supers = cfg["supers"]
    groups = [list(range(NC))]
    nt = NPC // P   # 49 node tiles per core

    f16 = mybir.dt.float16
    xT_d = nc.dram_tensor("xT", [IN_DIM, NPC], f16, kind="ExternalInput")
    isrc_d = nc.dram_tensor("isrc", [16, Mtot * 8], i16, kind="ExternalInput")
    idst_d = nc.dram_tensor("idst", [16, Mtot * 8], i16, kind="ExternalInput")
    edstl_d = nc.dram_tensor("edstl", [P, Mtot], bf16, kind="ExternalInput")
    wc1_d = nc.dram_tensor("wc1", [IN_DIM, 80], f32, kind="ExternalInput")
    wc2_d = nc.dram_tensor("wc2", [64, 66], f32, kind="ExternalInput")
    b1r_d = nc.dram_tensor("b1r", [W, 64], f32, kind="ExternalInput")
    b2r_d = nc.dram_tensor("b2r", [W, 64], f32, kind="ExternalInput")
    out_d = nc.dram_tensor("out", [NPC, OUT_DIM], bf16, kind="ExternalOutput")

    import os as _os
    _dbg = bool(_os.environ.get("K_DEBUG"))
    if _dbg:
        dbg1_d = nc.dram_tensor("dbg1", [NPC, EW], bf16, kind="ExternalOutput")
        dbg2_d = nc.dram_tensor("dbg2", [W, WPC * W], f32, kind="ExternalOutput")
    t1s_d = nc.dram_tensor("t1slice", [NPC, EW], bf16, kind="Internal")
    table1 = nc.dram_tensor("table1", [NP, EW], bf16, kind="Internal",
                            addr_space="Shared")
    t2s_d = nc.dram_tensor("t2slice", [NPC, EW], bf16, kind="Internal")
    table2 = nc.dram_tensor("table2", [NP, EW], bf16, kind="Internal",
                            addr_space="Shared")

    iota_d = nc.inline_tensor(
        np.tile(np.arange(W).astype(ml_dtypes.bfloat16), (P, 1)), "iotaw")
    ident_d = nc.inline_tensor(np.eye(W, dtype=np.float32), "identw")

    with tile.TileContext(nc) as tc:
        with tc.tile_pool(name="const", bufs=1) as cp, \
             tc.tile_pool(name="work", bufs=3) as wp, \
             tc.tile_pool(name="gath", bufs=2) as gp, \
             tc.tile_pool(name="dstg", bufs=1) as dgp, \
             tc.tile_pool(name="ohp", bufs=2) as op_, \
             tc.tile_pool(name="drain", bufs=3) as dp, \
             tc.tile_pool(name="eps", bufs=3, space="PSUM") as pp, \
             tc.tile_pool(name="nps", bufs=2, space="PSUM") as np_, \
             tc.tile_pool(name="tps", bufs=1, space="PSUM") as tpp:

            nc.gpsimd.load_library(library_config.mlp)

            iota = cp.tile([P, W], bf16, tag="iota")
            nc.scalar.dma_start(out=iota[:, :], in_=iota_d[:, :])
            ident = cp.tile([W, W], f32, tag="ident")
            nc.scalar.dma_start(out=ident[:, :], in_=ident_d[:, :])
            wc1 = cp.tile([IN_DIM, 80], f32, tag="wc1")
            nc.scalar.dma_start(out=wc1[:, :], in_=wc1_d[:, :])
            wc2 = cp.tile([64, 66], f32, tag="wc2")
            nc.scalar.dma_start(out=wc2[:, :], in_=wc2_d[:, :])
            b1r = cp.tile([W, 64], f32, tag="b1r")
            nc.scalar.dma_start(out=b1r[:, :], in_=b1r_d[:, :])
            b2r = cp.tile([W, 64], f32, tag="b2r")
            nc.scalar.dma_start(out=b2r[:, :], in_=b2r_d[:, :])
            edstl = cp.tile([P, Mtot], bf16, tag="edstl")
            nc.scalar.dma_start(out=edstl[:, :], in_=edstl_d[:, :])
            isrc = cp.tile([P, Mtot * 8], i16, tag="isrc")
            idst = cp.tile([P, Mtot * 8], i16, tag="idst")
            for k in range(8):
                nc.scalar.dma_start(out=isrc[16 * k:16 * (k + 1), :],
                                    in_=isrc_d[:, :])
                nc.scalar.dma_start(out=idst[16 * k:16 * (k + 1), :],
                                    in_=idst_d[:, :])
            h2big = cp.tile([W, WPC * W], f32, tag="h2big")

            # distinct num_idxs register values (chunks are <=8 blocks)
            regs = {}
            for S in supers:
                for run in (S["nA"], S["nB"], S["nA"] + S["nB"]):
                    left = run
                    while left > 0:
                        nb = min(8, left)
                        if nb * P not in regs:
                            regs[nb * P] = nc.gpsimd.to_reg(nb * P)
                        left -= nb

            def node_phase1():
                stage = cp.tile([P, nt * EW], bf16, tag="nstage")
                stf = stage[:, :].bitcast(f32)
                xth = cp.tile([IN_DIM, NPC], f16, tag="xth")
                nc.sync.dma_start(out=xth[:, :], in_=xT_d[:, :])
                for t in range(nt):
                    xt = wp.tile([IN_DIM, P], f32, tag="xt")
                    nc.vector.tensor_copy(out=xt[:, :],
                                          in_=xth[:, t * P:(t + 1) * P])
                    hp = np_.tile([P, 80], f32, tag="hps")
                    nc.tensor.matmul(out=hp[:, :], lhsT=xt[:, :],
                                     rhs=wc1[:, :], start=True, stop=True)
                    nc.vector.tensor_copy(
                        out=stage[:, t * EW:t * EW + 64], in_=hp[:, :64])
                    nc.vector.tensor_copy(
                        out=_sub(stf, t * (EW // 2) + 32, [[1, 16]]),
                        in_=hp[:, 64:80])
                # stage [p, t*EW+slot] -> t1s row t*128+p
                nc.sync.dma_start(
                    out=_dram_ap(t1s_d, 0,
                                 [[EW, P], [EW * P, nt], [1, EW]]),
                    in_=_sub(stage[:, :], 0, [[EW, nt], [1, EW]]))
                nc.gpsimd.collective_compute(
                    "AllGather", Alu.bypass, replica_groups=groups,
                    ins=[t1s_d[:, :]], outs=[table1[:, :]])

            def node_phase2():
                stage = cp.tile([P, nt * EW], bf16, tag="nstage")
                stf = stage[:, :].bitcast(f32)
                for t in range(nt):
                    tp2 = tpp.tile([W, P], f32, tag="tps")
                    nc.tensor.transpose(tp2[:, :W],
                                        h2big[:, (2 * t) * W:(2 * t + 1) * W],
                                        ident[:, :])
                    nc.tensor.transpose(tp2[:, W:],
                                        h2big[:, (2 * t + 1) * W:(2 * t + 2) * W],
                                        ident[:, :])
                    h2T = wp.tile([W, P], f32, tag="h2T")
                    nc.vector.tensor_copy(out=h2T[:, :], in_=tp2[:, :])
                    hp2 = np_.tile([P, 66], f32, tag="hp2s")
                    nc.tensor.matmul(out=hp2[:, :], lhsT=h2T[:, :],
                                     rhs=wc2[:, :], start=True, stop=True)
                    nc.vector.tensor_copy(
                        out=stage[:, t * EW:t * EW + 64], in_=hp2[:, :64])
                    nc.vector.tensor_copy(
                        out=_sub(stf, t * (EW // 2) + 32, [[1, 2]]),
                        in_=hp2[:, 64:66])
                nc.sync.dma_start(
                    out=_dram_ap(t2s_d, 0,
                                 [[EW, P], [EW * P, nt], [1, EW]]),
                    in_=_sub(stage[:, :], 0, [[EW, nt], [1, EW]]))
                nc.gpsimd.collective_compute(
                    "AllGather", Alu.bypass, replica_groups=groups,
                    ins=[t2s_d[:, :]], outs=[table2[:, :]])

            GMAX = 8    # max blocks (1024 idxs) per dma_gather (hw ring cap)

            def chunked_gather(dst_tile, dst_w, blk0, nblk, src_ap, idx_tile,
                               icol0):
                """gather nblk blocks into dst_tile cols [blk0*dst_w ...),
                chunks of <=GMAX blocks; idx cols start at icol0."""
                done = 0
                while done < nblk:
                    nb = min(GMAX, nblk - done)
                    _gather(nc.gpsimd,
                            _sub(dst_tile[:, :], (blk0 + done) * dst_w,
                                 [[dst_w, nb], [1, dst_w]]),
                            src_ap,
                            idx_tile[:, icol0 + 8 * done:
                                     icol0 + 8 * (done + nb)],
                            nb * P, dst_w, regs[nb * P])
                    done += nb

            def edge_phase(table, loc_table, H, GW_mm, dstoff, doff, layer):
                # dst gathers first: independent of the AllGather
                Dts = []
                for sp, S in enumerate(supers):
                    nblk = S["nA"] + S["nB"]
                    D = dgp.tile([P, nblk * DSTW], bf16, tag=f"D{sp}")
                    chunked_gather(D, DSTW, 0, nblk,
                                   loc_table[:, dstoff:dstoff + DSTW],
                                   idst, 8 * S["bb0"])
                    Dts.append(D)

                for sp, S in enumerate(supers):
                    sl0, nsl, bb0 = S["sl0"], S["nsl"], S["bb0"]
                    nAs, nBs = S["nA"], S["nB"]
                    nblk = nAs + nBs
                    G = gp.tile([P, nblk * SRCW], bf16, tag="G")
                    if nAs:
                        chunked_gather(G, SRCW, 0, nAs,
                                       table[:HALF, :SRCW], isrc, 8 * bb0)
                    if nBs:
                        chunked_gather(G, SRCW, nAs, nBs,
                                       table[HALF:NP, :SRCW], isrc,
                                       8 * (bb0 + nAs))
                    D = Dts[sp]
                    Gf = G[:, :].bitcast(f32)       # [P, nblk*40]
                    Df = D[:, :].bitcast(f32)       # [P, nblk*8]
                    e = wp.tile([P, nblk * H], f32, tag="e")
                    nc.vector.tensor_tensor(
                        out=_sub(e[:, :], 0, [[H, nblk], [1, H]]),
                        in0=_sub(Gf, 32, [[SRCW // 2, nblk], [1, H]]),
                        in1=_sub(Df, doff, [[DSTW // 2, nblk], [1, H]]),
                        op=Alu.add)
                    # lrelu = max(0.2*e, e); scalar-engine Lrelu ignores alpha
                    nc.vector.scalar_tensor_tensor(
                        out=e[:, :], in0=e[:, :], scalar=NEG_SLOPE,
                        in1=e[:, :], op0=Alu.mult, op1=Alu.max)
                    nc.scalar.activation(
                        out=_sub(G[:, :], 64, [[SRCW, nblk], [1, H]]),
                        in_=_sub(e[:, :], 0, [[H, nblk], [1, H]]),
                        func=Act.Exp)
                    oh = op_.tile([P, nblk * W], bf16, tag="oh")
                    nc.vector.tensor_tensor(
                        out=_sub(oh[:, :], 0, [[W, nblk], [1, W]]),
                        in0=_sub(iota[:, :], 0, [[0, nblk], [1, W]]),
                        in1=_sub(edstl[:, :], bb0, [[1, nblk], [0, W]]),
                        op=Alu.is_equal)
                    in0m = _sub(G[:, :], 0,
                                [[SRCW, nblk], [64 // H, H], [1, 64 // H]])
                    in1p = _sub(G[:, :], 64,
                                [[SRCW, nblk], [1, H], [0, 64 // H]])
                    nc.vector.tensor_tensor(out=in0m, in0=in0m, in1=in1p,
                                            op=Alu.mult)

                    stage = dp.tile([W, nsl * 64], f32, tag="stage")
                    for k in range(nsl):
                        pos = S["slotsA"][k] + S["slotsB"][k]
                        ps = pp.tile([W, GW_mm], f32, tag="ps")
                        for i, j in enumerate(pos):
                            nc.tensor.matmul(
                                out=ps[:, :],
                                lhsT=oh[:, j * W:(j + 1) * W],
                                rhs=G[:, j * SRCW:j * SRCW + GW_mm],
                                start=(i == 0), stop=(i == len(pos) - 1))
                        den = dp.tile([W, H], f32, tag="den")
                        nc.vector.tensor_scalar_add(den[:, :],
                                                    ps[:, 64:64 + H], 1e-10)
                        inv = dp.tile([W, H], f32, tag="inv")
                        nc.vector.reciprocal(inv[:, :], den[:, :])
                        nc.vector.tensor_tensor(
                            out=_sub(stage[:, :], k * 64,
                                     [[64 // H, H], [1, 64 // H]]),
                            in0=_sub(ps[:, :], 0,
                                     [[64 // H, H], [1, 64 // H]]),
                            in1=_sub(inv[:, :], 0, [[1, H], [0, 64 // H]]),
                            op=Alu.mult)
                    if layer == 1:
                        tb = dp.tile([W, nsl * 64], f32, tag="tb")
                        nc.vector.tensor_tensor(
                            out=tb[:, :], in0=stage[:, :],
                            in1=_sub(b1r[:, :], 0, [[0, nsl], [1, 64]]),
                            op=Alu.add)
                        ex = dp.tile([W, nsl * 64], f32, tag="ex")
                        nc.scalar.activation(out=ex[:, :], in_=tb[:, :],
                                             func=Act.Exp)
                        nc.vector.tensor_scalar(
                            out=ex[:, :], in0=ex[:, :], scalar1=-1.0,
                            scalar2=0.0, op0=Alu.add, op1=Alu.min)
                        rl = dp.tile([W, nsl * 64], f32, tag="rl")
                        nc.vector.tensor_scalar_max(rl[:, :], tb[:, :], 0.0)
                        nc.vector.tensor_tensor(
                            out=h2big[:, sl0 * 64:(sl0 + nsl) * 64],
                            in0=ex[:, :], in1=rl[:, :], op=Alu.add)
                    else:
                        ob = dp.tile([W, nsl * 64], bf16, tag="ob")
                        nc.vector.tensor_tensor(
                            out=ob[:, :], in0=stage[:, :],
                            in1=_sub(b2r[:, :], 0, [[0, nsl], [1, 64]]),
                            op=Alu.add)
                        # ob[woff, k*64+c] -> out row (sl0+k)*64+woff
                        nc.sync.dma_start(
                            out=_dram_ap(out_d, sl0 * W * 64,
                                         [[64, W], [W * 64, nsl], [1, 64]]),
                            in_=_sub(ob[:, :], 0, [[64, nsl], [1, 64]]))

            node_phase1()
            if _dbg:
                nc.sync.dma_start(out=dbg1_d[:, :], in_=t1s_d[:, :])
            edge_phase(table1, t1s_d, HEADS1, 72, 80, 0, layer=1)
            if _dbg:
                nc.sync.dma_start(out=dbg2_d[:, :], in_=h2big[:, :])
            node_phase2()
            edge_phase(table2, t2s_d, 1, 65, 64, 1, layer=2)


def kernel(**inputs):
    import concourse.bacc as bacc
    from concourse.bass_utils import run_bass_kernel_spmd

    n = inputs["x"].shape[0]
    cfg, in_maps, perm = _prep(**inputs)

    nc = bacc.Bacc("TRN2", target_bir_lowering=False, debug=False,
                   num_devices=NC)
    _build(nc, cfg)
    nc.compile()

    res = run_bass_kernel_spmd(nc, in_maps, core_ids=list(range(NC)))
    full = np.concatenate([np.asarray(r["out"]) for r in res.results], axis=0)
    out = full[perm[:n]].astype(np.float32)
    return np.ascontiguousarray(out)


# revision 4
# speedup vs baseline: 1.0177x; 1.0128x over previous
"""2-layer GAT on 8 Trainium2 NeuronCores — v2 (dma_gather batch gathers).

Design (vs v1 per-block indirect DMAs):
  - One fused table per layer, [NP, 128] bf16-typed 256B rows:
      L1 row: [h bf16 x64 | sS f32 x8 (slots 64..80) | sD f32 x8 (80..96) | pad]
      L2 row: [h bf16 x64 | sS f32 x1 (64..66) | sD f32 x1 (66..68) | pad]
    f32 scores are bit-embedded in the bf16 tensor and read via AP bitcast.
  - Nodes are slot-permuted (perm) so both layers share one set of edge
    indices; dst rows for a core's windows are its OWN rows, so the dst
    (sD) gather reads the core-local pre-collective table t{1,2}s_d with
    local int16 indices — no AllGather dependency, hoisted before it.
  - src gathers use dma_gather (int16 idx) split into A (<32768) / B runs
    of blocks per super; per-slot PSUM matmul accumulation as in v1.
  - Scores f32 end-to-end (f32 node matmul, f32 e/lrelu), p bf16.
  - Output bf16, converted to f32 on host.
"""

import numpy as np
import ml_dtypes

P = 128
W = 64
NC = 8
WPC = 98
NPC = WPC * W          # 6272
NP = NC * NPC          # 50176
HALF = 32768
IN_DIM = 128
HEADS1 = 8
OUT_DIM = 64
NEG_SLOPE = 0.2
SUPER_BLK = 72
SRCW = 80              # bf16 slots gathered per src row (h64 + sS f32x8)
DSTW = 16              # bf16 slots gathered per dst row (32B)
EW = 128               # table row width in bf16 slots (256B stride)


def _mk_head_mat(a):
    H, C = a.shape
    A = np.zeros((H * C, H), np.float32)
    for h in range(H):
        A[h * C:(h + 1) * C, h] = a[h]
    return A


def _wrap16(vals):
    """int16 index stream [n*128] -> wrapped [16, n*8] (i at [i%16, i//16])."""
    v = np.asarray(vals, np.int16)
    assert len(v) % 16 == 0
    return np.ascontiguousarray(v.reshape(-1, 16).T)


def _prep(x, edge_index, W1, a_src1, a_dst1, b1, W2, a_src2, a_dst2, b2):
    n = x.shape[0]
    assert n <= NP

    x = np.asarray(x, np.float32)
    ei = np.asarray(edge_index)
    src = np.concatenate([ei[0], np.arange(n)]).astype(np.int64)
    dst = np.concatenate([ei[1], np.arange(n)]).astype(np.int64)

    win = (dst // W).astype(np.int64)
    order = np.argsort(win, kind="stable")
    src, dst, win = src[order], dst[order], win[order]
    nw = NP // W
    counts = np.bincount(win, minlength=nw)
    starts = np.concatenate([[0], np.cumsum(counts)])

    counts_c = counts.reshape(NC, WPC)
    orders = [np.argsort(-counts_c[c], kind="stable") for c in range(NC)]

    # perm[node] = row in slot-ordered table (slot s of core c holds window
    # orders[c][s]);  pinv[row] = node
    perm = np.empty(NP, np.int64)
    for c in range(NC):
        inv = np.empty(WPC, np.int64)
        inv[orders[c]] = np.arange(WPC)
        wl = np.arange(WPC)
        base = (c * WPC + wl) * W
        for woff in range(W):
            perm[base + woff] = c * NPC + inv * W + woff
    pinv = np.empty(NP, np.int64)
    pinv[perm] = np.arange(NP)

    psrc = perm[src]
    pdst = perm[dst]
    isA = psrc < HALF

    # per (core, slot) A/B edge counts -> uniform KA/KB
    nA = np.zeros((NC, WPC), np.int64)
    nB = np.zeros((NC, WPC), np.int64)
    for c in range(NC):
        for s in range(WPC):
            wglob = c * WPC + orders[c][s]
            e0, e1 = starts[wglob], starts[wglob + 1]
            a = int(isA[e0:e1].sum())
            nA[c, s] = a
            nB[c, s] = (e1 - e0) - a
    KA = np.maximum(np.ceil(nA / P).astype(np.int64).max(axis=0), 0)
    KB = np.ceil(nB / P).astype(np.int64).max(axis=0)
    # every slot needs >=1 block for psum start/stop (self loops guarantee
    # edges exist, but they might all be in one half)
    KA = np.where(KA + KB == 0, 1, KA)
    Mtot = int((KA + KB).sum())

    # greedy supers
    supers = []   # dicts: sl0, nsl, bb0, nA (blocks), nB, slotsA, slotsB
    s = 0
    bb0 = 0
    while s < WPC:
        sl0 = s
        nblk = 0
        while s < WPC and nblk + KA[s] + KB[s] <= SUPER_BLK:
            nblk += int(KA[s] + KB[s])
            s += 1
        nsl = s - sl0
        nAs = int(KA[sl0:s].sum())
        nBs = int(KB[sl0:s].sum())
        slotsA, slotsB = [], []
        ao, bo = 0, nAs
        for k in range(nsl):
            slotsA.append(list(range(ao, ao + int(KA[sl0 + k]))))
            slotsB.append(list(range(bo, bo + int(KB[sl0 + k]))))
            ao += int(KA[sl0 + k])
            bo += int(KB[sl0 + k])
        supers.append(dict(sl0=sl0, nsl=nsl, bb0=bb0, nA=nAs, nB=nBs,
                           slotsA=slotsA, slotsB=slotsB))
        bb0 += nAs + nBs
    assert bb0 == Mtot

    # per-core streams
    in_maps = []
    for c in range(NC):
        isrc_cols = np.zeros((16, Mtot * 8), np.int16)
        idst_cols = np.zeros((16, Mtot * 8), np.int16)
        edstl = np.full((P, Mtot), W, np.uint8)
        for S in supers:
            sl0, nsl, b0 = S["sl0"], S["nsl"], S["bb0"]
            sA = np.zeros(S["nA"] * P, np.int64)   # src rel (A)
            sB = np.zeros(S["nB"] * P, np.int64)
            dA = np.zeros(S["nA"] * P, np.int64)   # dst rel (local)
            dB = np.zeros(S["nB"] * P, np.int64)
            lA = np.full(S["nA"] * P, W, np.int64)
            lB = np.full(S["nB"] * P, W, np.int64)
            ao = bo = 0
            for k in range(nsl):
                sidx = sl0 + k
                wglob = c * WPC + orders[c][sidx]
                e0, e1 = starts[wglob], starts[wglob + 1]
                m = isA[e0:e1]
                pse, pde, de = psrc[e0:e1], pdst[e0:e1], dst[e0:e1]
                ea, eb = m.nonzero()[0], (~m).nonzero()[0]
                na, nb_ = len(ea), len(eb)
                sA[ao:ao + na] = pse[ea]
                dA[ao:ao + na] = pde[ea] - c * NPC
                lA[ao:ao + na] = de[ea] - wglob * W
                sB[bo:bo + nb_] = pse[eb] - HALF
                dB[bo:bo + nb_] = pde[eb] - c * NPC
                lB[bo:bo + nb_] = de[eb] - wglob * W
                ao += int(KA[sidx]) * P
                bo += int(KB[sidx]) * P
            assert dA.max(initial=0) < NPC and dB.max(initial=0) < NPC
            stream_l = np.concatenate([lA, lB])
            nblk = S["nA"] + S["nB"]
            edstl[:, b0:b0 + nblk] = (
                stream_l.reshape(nblk, P).T.astype(np.uint8))
            isrc_cols[:, 8 * b0:8 * (b0 + S["nA"])] = _wrap16(sA)
            isrc_cols[:, 8 * (b0 + S["nA"]):8 * (b0 + nblk)] = _wrap16(sB)
            idst_cols[:, 8 * b0:8 * (b0 + nblk)] = _wrap16(
                np.concatenate([dA, dB]))

        xp = np.zeros((NPC, IN_DIM), np.float32)
        rows = pinv[c * NPC:(c + 1) * NPC]
        valid = rows < n
        xp[valid] = x[rows[valid]]
        in_maps.append({
            "xT": np.ascontiguousarray(xp.T.astype(np.float16)),
            "isrc": np.ascontiguousarray(isrc_cols),
            "idst": np.ascontiguousarray(idst_cols),
            "edstl": np.ascontiguousarray(edstl),
        })

    W1 = np.asarray(W1, np.float32)
    W2 = np.asarray(W2, np.float32)
    wc1 = np.concatenate(
        [W1, W1 @ _mk_head_mat(np.asarray(a_src1, np.float32)),
         W1 @ _mk_head_mat(np.asarray(a_dst1, np.float32))], axis=1)  # [128,80]
    wc2 = np.concatenate(
        [W2, W2 @ np.asarray(a_src2, np.float32).T,
         W2 @ np.asarray(a_dst2, np.float32).T], axis=1)              # [64,66]
    b1r = np.tile(np.asarray(b1, np.float32)[None, :], (W, 1))
    b2r = np.tile(np.asarray(b2, np.float32)[None, :], (W, 1))
    for m in in_maps:
        m["wc1"] = np.ascontiguousarray(wc1, np.float32)
        m["wc2"] = np.ascontiguousarray(wc2, np.float32)
        m["b1r"] = np.ascontiguousarray(b1r, np.float32)
        m["b2r"] = np.ascontiguousarray(b2r, np.float32)

    cfg = dict(Mtot=Mtot, supers=supers)
    return cfg, in_maps, perm


def _sub(apbase, off, dims):
    import concourse.bass as bass
    return bass.AP(tensor=apbase.tensor, offset=apbase.offset + off,
                   ap=[list(apbase.ap[0])] + [list(d) for d in dims])


def _dram_ap(tensor, off, dims):
    import concourse.bass as bass
    return bass.AP(tensor=tensor, offset=off, ap=[list(d) for d in dims])


def _gather(g, out_ap, in_ap, idxs_ap, num_idxs, elem_size, reg):
    """dma_gather without the elem_size%256 assert (sub-256B reads verified
    on hw)."""
    import concourse.mybir as mybir
    elem_step = in_ap.ap[0][0]
    stride_bytes = elem_step * mybir.dt.size(in_ap.dtype)
    assert stride_bytes % 256 == 0
    _in_ap = g.lower_ap_dma(in_ap, for_custom_bir_dma=True)
    _idxs_ap = g.lower_ap(idxs_ap)
    _out_ap = g.lower_ap(out_ap)
    return g.add_instruction(
        mybir.InstDMAGatherAnt(
            name=g.bass.get_next_instruction_name(),
            ins=[*_in_ap, _idxs_ap, g.lower_val_access(reg)],
            outs=[_out_ap],
            transpose=False, num_idxs=num_idxs, elem_size=elem_size,
            stride_bytes_256=stride_bytes // 256, gen_mode=0,
            single_packet=True, queue_num=0, sbuf_tokens_per_rank=0,
            sbuf_free_dim_per_rank=0, sbuf_free_dim_pad_per_rank=0,
            sbuf_byte_offset=0,
        )
    )


def _build(nc, cfg):
    import concourse.mybir as mybir
    import concourse.tile as tile
    import concourse.library_config as library_config

    f32 = mybir.dt.float32
    bf16 = mybir.dt.bfloat16
    i16 = mybir.dt.int16
    Alu = mybir.AluOpType
    Act = mybir.ActivationFunctionType

    Mtot = cfg["Mtot"]
    supers = cfg["supers"]
    groups = [list(range(NC))]
    nt = NPC // P   # 49 node tiles per core

    f16 = mybir.dt.float16
    xT_d = nc.dram_tensor("xT", [IN_DIM, NPC], f16, kind="ExternalInput")
    isrc_d = nc.dram_tensor("isrc", [16, Mtot * 8], i16, kind="ExternalInput")
    idst_d = nc.dram_tensor("idst", [16, Mtot * 8], i16, kind="ExternalInput")
    u8 = mybir.dt.uint8
    edstl_d = nc.dram_tensor("edstl", [P, Mtot], u8, kind="ExternalInput")
    wc1_d = nc.dram_tensor("wc1", [IN_DIM, 80], f32, kind="ExternalInput")
    wc2_d = nc.dram_tensor("wc2", [64, 66], f32, kind="ExternalInput")
    b1r_d = nc.dram_tensor("b1r", [W, 64], f32, kind="ExternalInput")
    b2r_d = nc.dram_tensor("b2r", [W, 64], f32, kind="ExternalInput")
    out_d = nc.dram_tensor("out", [NPC, OUT_DIM], bf16, kind="ExternalOutput")

    import os as _os
    _dbg = bool(_os.environ.get("K_DEBUG"))
    if _dbg:
        dbg1_d = nc.dram_tensor("dbg1", [NPC, EW], bf16, kind="ExternalOutput")
        dbg2_d = nc.dram_tensor("dbg2", [W, WPC * W], f32, kind="ExternalOutput")
    t1s_d = nc.dram_tensor("t1slice", [NPC, EW], bf16, kind="Internal")
    table1 = nc.dram_tensor("table1", [NP, EW], bf16, kind="Internal",
                            addr_space="Shared")
    t2s_d = nc.dram_tensor("t2slice", [NPC, EW], bf16, kind="Internal")
    table2 = nc.dram_tensor("table2", [NP, EW], bf16, kind="Internal",
                            addr_space="Shared")

    iota_d = nc.inline_tensor(
        np.tile(np.arange(W).astype(ml_dtypes.bfloat16), (P, 1)), "iotaw")
    ident_d = nc.inline_tensor(np.eye(W, dtype=np.float32), "identw")

    with tile.TileContext(nc) as tc:
        with tc.tile_pool(name="const", bufs=1) as cp, \
             tc.tile_pool(name="work", bufs=3) as wp, \
             tc.tile_pool(name="gath", bufs=2) as gp, \
             tc.tile_pool(name="dstg", bufs=1) as dgp, \
             tc.tile_pool(name="ohp", bufs=2) as op_, \
             tc.tile_pool(name="drain", bufs=3) as dp, \
             tc.tile_pool(name="eps", bufs=3, space="PSUM") as pp, \
             tc.tile_pool(name="nps", bufs=2, space="PSUM") as np_, \
             tc.tile_pool(name="tps", bufs=1, space="PSUM") as tpp:

            nc.gpsimd.load_library(library_config.mlp)

            iota = cp.tile([P, W], bf16, tag="iota")
            nc.scalar.dma_start(out=iota[:, :], in_=iota_d[:, :])
            ident = cp.tile([W, W], f32, tag="ident")
            nc.scalar.dma_start(out=ident[:, :], in_=ident_d[:, :])
            wc1 = cp.tile([IN_DIM, 80], f32, tag="wc1")
            nc.scalar.dma_start(out=wc1[:, :], in_=wc1_d[:, :])
            wc2 = cp.tile([64, 66], f32, tag="wc2")
            nc.scalar.dma_start(out=wc2[:, :], in_=wc2_d[:, :])
            b1r = cp.tile([W, 64], f32, tag="b1r")
            nc.scalar.dma_start(out=b1r[:, :], in_=b1r_d[:, :])
            b2r = cp.tile([W, 64], f32, tag="b2r")
            nc.scalar.dma_start(out=b2r[:, :], in_=b2r_d[:, :])
            edstl8 = cp.tile([P, Mtot], u8, tag="edstl8")
            nc.scalar.dma_start(out=edstl8[:, :], in_=edstl_d[:, :])
            edstl = cp.tile([P, Mtot], bf16, tag="edstl")
            nc.vector.tensor_copy(out=edstl[:, :], in_=edstl8[:, :])
            isrc = cp.tile([P, Mtot * 8], i16, tag="isrc")
            idst = cp.tile([P, Mtot * 8], i16, tag="idst")
            for k in range(8):
                nc.scalar.dma_start(out=isrc[16 * k:16 * (k + 1), :],
                                    in_=isrc_d[:, :])
                nc.scalar.dma_start(out=idst[16 * k:16 * (k + 1), :],
                                    in_=idst_d[:, :])
            h2big = cp.tile([W, WPC * W], f32, tag="h2big")

            # distinct num_idxs register values (chunks are <=8 blocks)
            regs = {}
            for S in supers:
                for run in (S["nA"], S["nB"], S["nA"] + S["nB"]):
                    left = run
                    while left > 0:
                        nb = min(8, left)
                        if nb * P not in regs:
                            regs[nb * P] = nc.gpsimd.to_reg(nb * P)
                        left -= nb

            def node_phase1():
                stage = cp.tile([P, nt * EW], bf16, tag="nstage")
                stf = stage[:, :].bitcast(f32)
                xth = cp.tile([IN_DIM, NPC], f16, tag="xth")
                nc.sync.dma_start(out=xth[:, :], in_=xT_d[:, :])
                for t in range(nt):
                    xt = wp.tile([IN_DIM, P], f32, tag="xt")
                    nc.vector.tensor_copy(out=xt[:, :],
                                          in_=xth[:, t * P:(t + 1) * P])
                    hp = np_.tile([P, 80], f32, tag="hps")
                    nc.tensor.matmul(out=hp[:, :], lhsT=xt[:, :],
                                     rhs=wc1[:, :], start=True, stop=True)
                    nc.vector.tensor_copy(
                        out=stage[:, t * EW:t * EW + 64], in_=hp[:, :64])
                    nc.vector.tensor_copy(
                        out=_sub(stf, t * (EW // 2) + 32, [[1, 16]]),
                        in_=hp[:, 64:80])
                # stage [p, t*EW+slot] -> t1s row t*128+p
                nc.sync.dma_start(
                    out=_dram_ap(t1s_d, 0,
                                 [[EW, P], [EW * P, nt], [1, EW]]),
                    in_=_sub(stage[:, :], 0, [[EW, nt], [1, EW]]))
                nc.gpsimd.collective_compute(
                    "AllGather", Alu.bypass, replica_groups=groups,
                    ins=[t1s_d[:, :]], outs=[table1[:, :]])

            def node_phase2():
                stage = cp.tile([P, nt * EW], bf16, tag="nstage")
                stf = stage[:, :].bitcast(f32)
                for t in range(nt):
                    tp2 = tpp.tile([W, P], f32, tag="tps")
                    nc.tensor.transpose(tp2[:, :W],
                                        h2big[:, (2 * t) * W:(2 * t + 1) * W],
                                        ident[:, :])
                    nc.tensor.transpose(tp2[:, W:],
                                        h2big[:, (2 * t + 1) * W:(2 * t + 2) * W],
                                        ident[:, :])
                    h2T = wp.tile([W, P], f32, tag="h2T")
                    nc.vector.tensor_copy(out=h2T[:, :], in_=tp2[:, :])
                    hp2 = np_.tile([P, 66], f32, tag="hp2s")
                    nc.tensor.matmul(out=hp2[:, :], lhsT=h2T[:, :],
                                     rhs=wc2[:, :], start=True, stop=True)
                    nc.vector.tensor_copy(
                        out=stage[:, t * EW:t * EW + 64], in_=hp2[:, :64])
                    nc.vector.tensor_copy(
                        out=_sub(stf, t * (EW // 2) + 32, [[1, 2]]),
                        in_=hp2[:, 64:66])
                nc.sync.dma_start(
                    out=_dram_ap(t2s_d, 0,
                                 [[EW, P], [EW * P, nt], [1, EW]]),
                    in_=_sub(stage[:, :], 0, [[EW, nt], [1, EW]]))
                nc.gpsimd.collective_compute(
                    "AllGather", Alu.bypass, replica_groups=groups,
                    ins=[t2s_d[:, :]], outs=[table2[:, :]])

            GMAX = 8    # max blocks (1024 idxs) per dma_gather (hw ring cap)

            def chunked_gather(dst_tile, dst_w, blk0, nblk, src_ap, idx_tile,
                               icol0):
                """gather nblk blocks into dst_tile cols [blk0*dst_w ...),
                chunks of <=GMAX blocks; idx cols start at icol0."""
                done = 0
                while done < nblk:
                    nb = min(GMAX, nblk - done)
                    _gather(nc.gpsimd,
                            _sub(dst_tile[:, :], (blk0 + done) * dst_w,
                                 [[dst_w, nb], [1, dst_w]]),
                            src_ap,
                            idx_tile[:, icol0 + 8 * done:
                                     icol0 + 8 * (done + nb)],
                            nb * P, dst_w, regs[nb * P])
                    done += nb

            def edge_phase(table, loc_table, H, GW_mm, dstoff, doff, layer):
                # dst gathers first: independent of the AllGather
                Dts = []
                for sp, S in enumerate(supers):
                    nblk = S["nA"] + S["nB"]
                    D = dgp.tile([P, nblk * DSTW], bf16, tag=f"D{sp}")
                    chunked_gather(D, DSTW, 0, nblk,
                                   loc_table[:, dstoff:dstoff + DSTW],
                                   idst, 8 * S["bb0"])
                    Dts.append(D)

                for sp, S in enumerate(supers):
                    sl0, nsl, bb0 = S["sl0"], S["nsl"], S["bb0"]
                    nAs, nBs = S["nA"], S["nB"]
                    nblk = nAs + nBs
                    G = gp.tile([P, nblk * SRCW], bf16, tag="G")
                    if nAs:
                        chunked_gather(G, SRCW, 0, nAs,
                                       table[:HALF, :SRCW], isrc, 8 * bb0)
                    if nBs:
                        chunked_gather(G, SRCW, nAs, nBs,
                                       table[HALF:NP, :SRCW], isrc,
                                       8 * (bb0 + nAs))
                    D = Dts[sp]
                    Gf = G[:, :].bitcast(f32)       # [P, nblk*40]
                    Df = D[:, :].bitcast(f32)       # [P, nblk*8]
                    e = wp.tile([P, nblk * H], f32, tag="e")
                    nc.vector.tensor_tensor(
                        out=_sub(e[:, :], 0, [[H, nblk], [1, H]]),
                        in0=_sub(Gf, 32, [[SRCW // 2, nblk], [1, H]]),
                        in1=_sub(Df, doff, [[DSTW // 2, nblk], [1, H]]),
                        op=Alu.add)
                    # lrelu = max(0.2*e, e); scalar-engine Lrelu ignores alpha
                    nc.vector.scalar_tensor_tensor(
                        out=e[:, :], in0=e[:, :], scalar=NEG_SLOPE,
                        in1=e[:, :], op0=Alu.mult, op1=Alu.max)
                    nc.scalar.activation(
                        out=_sub(G[:, :], 64, [[SRCW, nblk], [1, H]]),
                        in_=_sub(e[:, :], 0, [[H, nblk], [1, H]]),
                        func=Act.Exp)
                    oh = op_.tile([P, nblk * W], bf16, tag="oh")
                    nc.vector.tensor_tensor(
                        out=_sub(oh[:, :], 0, [[W, nblk], [1, W]]),
                        in0=_sub(iota[:, :], 0, [[0, nblk], [1, W]]),
                        in1=_sub(edstl[:, :], bb0, [[1, nblk], [0, W]]),
                        op=Alu.is_equal)
                    in0m = _sub(G[:, :], 0,
                                [[SRCW, nblk], [64 // H, H], [1, 64 // H]])
                    in1p = _sub(G[:, :], 64,
                                [[SRCW, nblk], [1, H], [0, 64 // H]])
                    nc.vector.tensor_tensor(out=in0m, in0=in0m, in1=in1p,
                                            op=Alu.mult)

                    stage = dp.tile([W, nsl * 64], f32, tag="stage")
                    for k in range(nsl):
                        pos = S["slotsA"][k] + S["slotsB"][k]
                        ps = pp.tile([W, GW_mm], f32, tag="ps")
                        for i, j in enumerate(pos):
                            nc.tensor.matmul(
                                out=ps[:, :],
                                lhsT=oh[:, j * W:(j + 1) * W],
                                rhs=G[:, j * SRCW:j * SRCW + GW_mm],
                                start=(i == 0), stop=(i == len(pos) - 1))
                        den = dp.tile([W, H], f32, tag="den")
                        nc.vector.tensor_scalar_add(den[:, :],
                                                    ps[:, 64:64 + H], 1e-10)
                        inv = dp.tile([W, H], f32, tag="inv")
                        nc.vector.reciprocal(inv[:, :], den[:, :])
                        nc.vector.tensor_tensor(
                            out=_sub(stage[:, :], k * 64,
                                     [[64 // H, H], [1, 64 // H]]),
                            in0=_sub(ps[:, :], 0,
                                     [[64 // H, H], [1, 64 // H]]),
                            in1=_sub(inv[:, :], 0, [[1, H], [0, 64 // H]]),
                            op=Alu.mult)
                    if layer == 1:
                        tb = dp.tile([W, nsl * 64], f32, tag="tb")
                        nc.vector.tensor_tensor(
                            out=tb[:, :], in0=stage[:, :],
                            in1=_sub(b1r[:, :], 0, [[0, nsl], [1, 64]]),
                            op=Alu.add)
                        ex = dp.tile([W, nsl * 64], f32, tag="ex")
                        nc.scalar.activation(out=ex[:, :], in_=tb[:, :],
                                             func=Act.Exp)
                        nc.vector.tensor_scalar(
                            out=ex[:, :], in0=ex[:, :], scalar1=-1.0,
                            scalar2=0.0, op0=Alu.add, op1=Alu.min)
                        rl = dp.tile([W, nsl * 64], f32, tag="rl")
                        nc.vector.tensor_scalar_max(rl[:, :], tb[:, :], 0.0)
                        nc.vector.tensor_tensor(
                            out=h2big[:, sl0 * 64:(sl0 + nsl) * 64],
                            in0=ex[:, :], in1=rl[:, :], op=Alu.add)
                    else:
                        ob = dp.tile([W, nsl * 64], bf16, tag="ob")
                        nc.vector.tensor_tensor(
                            out=ob[:, :], in0=stage[:, :],
                            in1=_sub(b2r[:, :], 0, [[0, nsl], [1, 64]]),
                            op=Alu.add)
                        # ob[woff, k*64+c] -> out row (sl0+k)*64+woff
                        nc.sync.dma_start(
                            out=_dram_ap(out_d, sl0 * W * 64,
                                         [[64, W], [W * 64, nsl], [1, 64]]),
                            in_=_sub(ob[:, :], 0, [[64, nsl], [1, 64]]))

            node_phase1()
            if _dbg:
                nc.sync.dma_start(out=dbg1_d[:, :], in_=t1s_d[:, :])
            edge_phase(table1, t1s_d, HEADS1, 72, 80, 0, layer=1)
            if _dbg:
                nc.sync.dma_start(out=dbg2_d[:, :], in_=h2big[:, :])
            node_phase2()
            edge_phase(table2, t2s_d, 1, 65, 64, 1, layer=2)


def kernel(**inputs):
    import concourse.bacc as bacc
    from concourse.bass_utils import run_bass_kernel_spmd

    n = inputs["x"].shape[0]
    cfg, in_maps, perm = _prep(**inputs)

    nc = bacc.Bacc("TRN2", target_bir_lowering=False, debug=False,
                   num_devices=NC)
    _build(nc, cfg)
    nc.compile()

    res = run_bass_kernel_spmd(nc, in_maps, core_ids=list(range(NC)))
    full = np.concatenate([np.asarray(r["out"]) for r in res.results], axis=0)
    out = full[perm[:n]].astype(np.float32)
    return np.ascontiguousarray(out)
